# revision 58
# baseline (speedup 1.0000x reference)
import sys

if "/opt/trn_rl_repo" not in sys.path:
    sys.path.insert(0, "/opt/trn_rl_repo")

import numpy as np
from ml_dtypes import bfloat16 as np_bf16
from ml_dtypes import float8_e4m3fn as np_f8
import concourse.bacc as bacc
import concourse.bass as bass
import concourse.mybir as mybir
import concourse.tile as tile
from concourse.bass_utils import run_bass_kernel_spmd
from concourse.masks import make_identity

# Problem dims (hardcoded per spec)
DIM = 2048
DMEDIA = 1024
HEADS = 16
DH = 64
INNER = 1024
FF = 8192
LAT = 64
B = 4
NTOK = 2048
T = 1024          # tokens per core (one batch element, half its tokens)
P = 128
EPS = 1e-5
NCORES = 8

DC = DIM // P      # 16 dim chunks
IC = INNER // P    # 8 inner chunks
MC = DMEDIA // P   # 8 media-dim chunks
FC = FF // P       # 64 ffn chunks
TS = T // P        # 8 token sub-tiles
SCALE = DH ** -0.5

F32 = mybir.dt.float32
BF16 = mybir.dt.bfloat16
F8 = mybir.dt.float8e4
W8SCALE = 32.0
AF = mybir.ActivationFunctionType


def build_program():
    nc = bacc.Bacc("TRN2", target_bir_lowering=False, debug=False)

    x_d = nc.dram_tensor("x", [T, DIM], BF16, kind="ExternalInput")
    mediaT_d = nc.dram_tensor("mediaT", [P, MC * LAT], BF16, kind="ExternalInput")
    masklog_d = nc.dram_tensor("masklog", [LAT, 1], F32, kind="ExternalInput")
    wq_d = nc.dram_tensor("Wq", [8 * P, 2 * INNER], F8, kind="ExternalInput")
    wk_d = nc.dram_tensor("Wk", [IC * P, MC * P], BF16, kind="ExternalInput")
    wv_d = nc.dram_tensor("Wv", [2 * P, MC * 512], BF16, kind="ExternalInput")
    wo_d = nc.dram_tensor("Wo", [P, 4 * 2 * DIM], F8, kind="ExternalInput")
    w1_d = nc.dram_tensor("W1", [FC * P, 2048], F8, kind="ExternalInput")
    w2_d = nc.dram_tensor("W2", [128 * P, 1024], F8, kind="ExternalInput")
    g1s_d = nc.dram_tensor("g1s", [P, DC], F32, kind="ExternalInput")
    b1s_d = nc.dram_tensor("b1s", [P, DC], F32, kind="ExternalInput")
    g2_d = nc.dram_tensor("g2", [P, DC], F32, kind="ExternalInput")
    b2_d = nc.dram_tensor("b2", [P, DC], F32, kind="ExternalInput")
    tg1_d = nc.dram_tensor("tg1", [1, 1], F32, kind="ExternalInput")  # tanh(attn_gate)
    tg2_d = nc.dram_tensor("tg2", [1, 1], F32, kind="ExternalInput")  # tanh(ff_gate)
    sumsel_d = nc.dram_tensor("sumsel", [P, 2], BF16, kind="ExternalInput")
    onehot_d = nc.dram_tensor("onehot", [2, P], BF16, kind="ExternalInput")
    out_d = nc.dram_tensor("out", [T, DIM], F32, kind="ExternalOutput")

    from contextlib import ExitStack

    with tile.TileContext(nc) as tc, ExitStack() as es_pp:
        pp = es_pp.enter_context(tc.tile_pool(name="persist", bufs=1))
        ident = pp.tile([P, P], F32)
        make_identity(nc, ident)
        identb = pp.tile([P, P], BF16)
        nc.vector.tensor_copy(identb[:], ident[:])
        eps_sb = pp.tile([P, 1], F32)
        nc.vector.memset(eps_sb, EPS)
        tg1_sb = pp.tile([P, 1], F32)
        tg2_sb = pp.tile([P, 1], F32)
        mask_sb = pp.tile([P, 1], F32)  # masklog replicated on both halves
        # ln gains/biases as [P, DC]: element (p, c) = g[c*128+p]
        # (attention scale already folded into g1s/b1s on host)
        g1s_sb = pp.tile([P, DC], F32)
        b1s_sb = pp.tile([P, DC], F32)
        g2_sb = pp.tile([P, DC], F32)
        b2_sb = pp.tile([P, DC], F32)
        # col0: ones on partitions 0-63, col1: ones on partitions 64-127
        sumsel2 = pp.tile([P, 2], BF16)
        # row0 -> broadcast into cols 0-63, row1 -> cols 64-127
        onehot2 = pp.tile([2, P], BF16)
        onehot2f = pp.tile([2, P], F32)  # f32 twin for f32-moving matmuls

        def load_consts():
            # emitted after phase A so these dispatches don't sit ahead of
            # the latency-critical LN chain in the ACT queue
            nc.scalar.dma_start(tg1_sb[:], bass.AP(
                tensor=tg1_d.ap().tensor, offset=0, ap=[[0, P], [1, 1]]))
            nc.scalar.dma_start(tg2_sb[:], bass.AP(
                tensor=tg2_d.ap().tensor, offset=0, ap=[[0, P], [1, 1]]))
            nc.scalar.dma_start(mask_sb[0:LAT, :], masklog_d[:])
            nc.scalar.dma_start(mask_sb[LAT:P, :], masklog_d[:])
            nc.scalar.dma_start(g1s_sb[:], g1s_d[:])
            nc.scalar.dma_start(b1s_sb[:], b1s_d[:])
            nc.scalar.dma_start(g2_sb[:], g2_d[:])
            nc.scalar.dma_start(b2_sb[:], b2_d[:])
            nc.scalar.dma_start(sumsel2[:], sumsel_d[:])
            nc.scalar.dma_start(onehot2[:], onehot_d[:])
            nc.vector.tensor_copy(onehot2f[:], onehot2[:])

        # ---- Long-lived pools (properly nested open/close) -------------
        es_qn2 = ExitStack()    # qn2T8: EF..end of FFN2
        qn2Tp = es_qn2.enter_context(tc.tile_pool(name="qn2T_pool", bufs=1))
        qn2T8 = [qn2Tp.tile([P, 2, T], F8, tag=f"qn2T8_{i}",
                            name=f"qn2T8_{i}")
                 for i in range(DC // 2)]
        es_kv = ExitStack()     # kT/v2: phase A..end of attention
        kvp = es_kv.enter_context(tc.tile_pool(name="kv_pool", bufs=1))

        # ---------------- Phase A: media^T, K/V projections -------------
        kT_sb = kvp.tile([P, IC, LAT], BF16)            # k^T [inner, lat]
        # v packed per parity: v2_sb[(h%2)*64+lat, 4*(h//8)+(h%8)//2, dh]
        v2_sb = kvp.tile([P, IC, DH], BF16)

        with tc.tile_pool(name="ps_a", bufs=2, space="PSUM") as ps_a, \
             tc.tile_pool(name="media_p", bufs=1) as mp, \
             tc.tile_pool(name="wk_st", bufs=4) as wk_st, \
             tc.tile_pool(name="wv_st", bufs=2) as wv_st:
            mediaT = mp.tile([P, MC, LAT], BF16)   # media^T (host-prepped)
            nc.scalar.dma_start(
                mediaT[:], mediaT_d[:].rearrange("p (mc l) -> p mc l", l=LAT))
            # k^T: per inner chunk, accumulate over media-dim chunks
            for ic in range(IC):
                wk = wk_st.tile([P, MC * P], BF16, tag="wk")
                nc.scalar.dma_start(wk[:], wk_d[ic * P : (ic + 1) * P, :])
                pk = ps_a.tile([P, 512], F32, tag="psa")
                for mc in range(MC):
                    nc.tensor.matmul(
                        pk[:, :LAT], wk[:, mc * P : (mc + 1) * P],
                        mediaT[:, mc, :],
                        start=(mc == 0), stop=(mc == MC - 1))
                nc.vector.tensor_copy(kT_sb[:, ic, :], pk[:, :LAT])
            # v packed by parity: heads h%2==parity at partition base
            # parity*64, slot 4*half+g where h = 8*half + 2g + parity
            for half in range(2):
                wv = wv_st.tile([P, MC, 512], BF16, tag="wv")
                nc.scalar.dma_start(
                    wv[:],
                    wv_d[half * P : (half + 1) * P, :].rearrange(
                        "p (mc i) -> p mc i", i=512))
                for parity in range(2):
                    po = parity * LAT
                    pv = ps_a.tile([P, 512], F32, tag="psa")
                    for g in range(4):
                        for mc in range(MC):
                            nc.tensor.matmul(
                                pv[po : po + LAT, g * DH : (g + 1) * DH],
                                mediaT[:, mc, :],
                                wv[:, mc,
                                   g * 2 * DH + parity * DH :
                                   g * 2 * DH + (parity + 1) * DH],
                                start=(mc == 0), stop=(mc == MC - 1))
                    nc.vector.tensor_copy(
                        v2_sb[po : po + LAT, 4 * half : 4 * half + 4, :],
                        pv[po : po + LAT, :256].rearrange(
                            "l (g q) -> l g q", q=DH))

        load_consts()

        # ---- Front pipeline pools (persist across both token halves) --
        es_wo = ExitStack()
        wop = es_wo.enter_context(tc.tile_pool(name="wo_pool", bufs=1))
        wo_sb = wop.tile([P, 4, 2, DIM], F8, tag="wo")
        es_ao = ExitStack()
        aop = es_ao.enter_context(tc.tile_pool(name="ao_pool", bufs=2))
        es_qa = ExitStack()
        qap = es_qa.enter_context(tc.tile_pool(name="qa_pool", bufs=1))
        es_qnT = ExitStack()
        qnTp = es_qnT.enter_context(tc.tile_pool(name="qnT_pool", bufs=2))
        es_bw = ExitStack()     # B-phase working pools: outer so the second
        # half's LN work overlaps the first half's attention with no
        # pool-release dependencies
        xlp = es_bw.enter_context(tc.tile_pool(name="xload", bufs=6))
        qntp = es_bw.enter_context(tc.tile_pool(name="qn_t", bufs=4))
        stp = es_bw.enter_context(tc.tile_pool(name="stats", bufs=8))
        ps_tr = es_bw.enter_context(
            tc.tile_pool(name="ps_tr", bufs=2, space="PSUM"))

        HD = DIM // 2
        qnT2 = [None, None]
        for th in range(2):
            # ---- B(th): LN1 + transpose -> qnT8 (fp8, c8-pair packed) ---
            qnT = [qnTp.tile([P, 2, 512], F8, tag=f"qnT8_{c}",
                             name=f"qnT8_{c}_{th}") for c in range(DC // 2)]
            qnT2[th] = qnT
            for gg in range(2):
                grp = th * 2 + gg
                qts = []
                for i2 in range(2):
                    i = grp * 2 + i2
                    xhs = []
                    st = stp.tile([P, 4, 6], F32, tag="st")
                    for hf in range(2):
                        xh = xlp.tile([P, HD], BF16, tag="x")
                        nc.sync.dma_start(
                            xh[:], x_d[i * P : (i + 1) * P,
                                       hf * HD : (hf + 1) * HD])
                        for j in range(2):
                            nc.vector.bn_stats(
                                st[:, 2 * hf + j, :],
                                xh[:, j * 512 : (j + 1) * 512])
                        xhs.append(xh)
                    mv = stp.tile([P, 2], F32, tag="mv")
                    nc.vector.bn_aggr(mv[:], st[:])
                    rstd = stp.tile([P, 1], F32, tag="rstd")
                    nc.scalar.activation(
                        rstd[:], mv[:, 1:2], AF.Sqrt, bias=eps_sb[:])
                    nc.vector.reciprocal(rstd[:], rstd[:])
                    # center+scale on ACT: (x-mu)*rstd = rstd*x + (-mu*rstd)
                    nmr = stp.tile([P, 1], F32, tag="nmr")
                    nc.vector.tensor_mul(nmr[:], mv[:, 0:1], rstd[:])
                    nc.vector.tensor_scalar_mul(nmr[:], nmr[:], -1.0)
                    qt = qntp.tile([P, DIM], BF16, tag="qn")
                    for hf in range(2):
                        nc.scalar.activation(
                            qt[:, hf * HD : (hf + 1) * HD], xhs[hf][:],
                            AF.Identity, bias=nmr[:], scale=rstd[:])
                    qts.append(qt)
                for c in range(DC):
                    pt = ps_tr.tile([P, 256], BF16, tag="tr")
                    for i2 in range(2):
                        nc.tensor.transpose(
                            pt[:, i2 * P : (i2 + 1) * P],
                            qts[i2][:, c * P : (c + 1) * P], identb[:])
                    dst = qnT[c // 2][:, c % 2, gg * 256 : (gg + 1) * 256]
                    with nc.allow_low_precision(reason="fp8 q-proj input"):
                        if c % 2 == 0:
                            nc.scalar.activation(
                                dst, pt[:],
                                AF.Identity, bias=b1s_sb[:, c : c + 1],
                                scale=g1s_sb[:, c : c + 1])
                        else:
                            nc.vector.tensor_scalar(
                                dst, pt[:],
                                scalar1=g1s_sb[:, c : c + 1],
                                scalar2=b1s_sb[:, c : c + 1],
                                op0=mybir.AluOpType.mult,
                                op1=mybir.AluOpType.add)

        for th in range(2):
            qnT = qnT2[th]
            # ---- C(th): Q projection (two 4-bank passes) ----------------
            qT = [qap.tile([P, 512], BF16, tag=f"qT{i}", name=f"qT{i}_{th}")
                  for i in range(IC)]
            attnT = [qap.tile([P, 512], BF16, tag=f"aT{i}",
                              name=f"attnT{i}_{th}") for i in range(IC)]
            with tc.tile_pool(name="wq_st", bufs=6) as wqst, \
                 tc.tile_pool(name="ps_q", bufs=4, space="PSUM") as ps_q:
                for icg in range(2):
                    pqs = [ps_q.tile([P, 512], F32, tag="q", name=f"pq{i}")
                           for i in range(4)]
                    for c8 in range(DC // 2):
                        wq8t = wqst.tile([P, 2, 2 * 512], F8, tag="wq")
                        nc.sync.dma_start(
                            wq8t[:],
                            wq_d[c8 * P : (c8 + 1) * P, :].rearrange(
                                "p (r q) -> p r q", q=1024))
                        for i4 in range(4):
                            oc = icg * 512 + i4 * P
                            nc.tensor.matmul(
                                pqs[i4], wq8t[:, :, oc : oc + P],
                                qnT[c8][:],
                                start=(c8 == 0), stop=(c8 == DC // 2 - 1),
                                perf_mode=mybir.MatmulPerfMode.DoubleRow)
                    for i4 in range(4):
                        ic = icg * 4 + i4
                        if ic % 2 == 0:
                            nc.scalar.mul(qT[ic][:], pqs[i4],
                                          SCALE / W8SCALE)
                        else:
                            nc.vector.tensor_scalar_mul(
                                qT[ic][:], pqs[i4], SCALE / W8SCALE)

            if th == 0:
                # Wo load on the ACT HWDGE ring, overlapping attention
                nc.scalar.dma_start(
                    wo_sb[:], wo_d[:].rearrange(
                        "p (i8 r d) -> p i8 r d", r=2, d=DIM))

            # ---- D(th): attention --------------------------------------
            attn_oT = [aop.tile([P, 2, 512], F8, tag=f"ao{i}",
                                name=f"attn_oT8_{i}_{th}")
                       for i in range(IC // 2)]
            with tc.tile_pool(name="ps_at", bufs=3, space="PSUM") as ps_at:
                for ic in range(IC):
                    ps = ps_at.tile([P, 512], F32, tag="at")
                    for parity in range(2):
                        po = parity * LAT
                        nc.tensor.matmul(
                            ps[po : po + LAT, :],
                            kT_sb[po : po + LAT, ic, :],
                            qT[ic][po : po + LAT, :],
                            start=True, stop=True)
                    # exp(sim + masklog) fused on ACT
                    nc.scalar.activation(
                        attnT[ic][:], ps[:], AF.Exp, bias=mask_sb[:])

            # softmax denominators via ACT ln/exp; AV runs on the
            # UNNORMALIZED attnT and 1/sum is applied at PSUM evacuation
            with tc.tile_pool(name="ps_s2", bufs=2, space="PSUM") as ps_s2, \
                 tc.tile_pool(name="ps_b", bufs=2, space="PSUM") as ps_b, \
                 tc.tile_pool(name="ps_av", bufs=2, space="PSUM") as ps_av, \
                 tc.tile_pool(name="rp_pool", bufs=3) as rpp:
                for ic in range(IC):
                    # rows 0/1 = sumexp of heads 2ic / 2ic+1
                    ps2 = ps_s2.tile([2, 512], F32, tag="s2")
                    nc.tensor.matmul(
                        ps2[:], sumsel2[:], attnT[ic][:],
                        start=True, stop=True)
                    s2ln = rpp.tile([2, 512], F32, tag="s2ln")
                    nc.scalar.activation(s2ln[:], ps2[:], AF.Ln)
                    pb = ps_b.tile([P, 512], F32, tag="b")
                    nc.tensor.matmul(
                        pb[:], onehot2f[:], s2ln[:], start=True, stop=True)
                    rp = rpp.tile([P, 512], BF16, tag="rp")
                    with nc.allow_low_precision(reason="softmax denom bf16"):
                        nc.scalar.activation(rp[:], pb[:], AF.Exp, scale=-1.0)
                    pav = ps_av.tile([P, 512], F32, tag="av")
                    for hh in range(2):
                        h = ic * 2 + hh
                        po = hh * LAT
                        vslot = 4 * (h // 8) + (h % 8) // 2
                        nc.tensor.matmul(
                            pav[po : po + LAT, :],
                            v2_sb[po : po + LAT, vslot, :],
                            attnT[ic][po : po + LAT, :],
                            start=True, stop=True)
                    with nc.allow_low_precision(reason="fp8 o-proj in"):
                        nc.vector.tensor_mul(
                            attn_oT[ic // 2][:, ic % 2, :], pav[:], rp[:])

            # ---- EF(th): O-proj + residual, LN2, -> qn2T8, x1 -> out_d --
            with tc.tile_pool(name="xstr", bufs=4) as xstr, \
                 tc.tile_pool(name="x1t", bufs=3) as x1p, \
                 tc.tile_pool(name="qn2_t", bufs=3) as qn2tp, \
                 tc.tile_pool(name="stats2", bufs=8) as st2p, \
                 tc.tile_pool(name="ps_o", bufs=4, space="PSUM") as ps_o:
                for gg in range(2):
                    grp = th * 2 + gg
                    q2ts = []
                    for t2 in range(2):
                        ts_ = grp * 2 + t2
                        x1t = x1p.tile([P, DIM], F32, tag="x1")
                        for dc4 in range(4):
                            sl = slice(dc4 * 512, (dc4 + 1) * 512)
                            po_ = ps_o.tile([P, 512], F32, tag="o")
                            for i8 in range(IC // 2):
                                nc.tensor.matmul(
                                    po_[:],
                                    attn_oT[i8][:, :, (ts_ % 4) * P :
                                                (ts_ % 4 + 1) * P],
                                    wo_sb[:, i8, :, sl],
                                    start=(i8 == 0),
                                    stop=(i8 == IC // 2 - 1),
                                    perf_mode=mybir.MatmulPerfMode.DoubleRow)
                            nc.scalar.mul(x1t[:, sl], po_[:], tg1_sb[:])
                            xc = xstr.tile([P, 512], BF16, tag="xc")
                            nc.sync.dma_start(
                                xc[:], x_d[ts_ * P : (ts_ + 1) * P, sl])
                            nc.vector.tensor_add(
                                x1t[:, sl], x1t[:, sl], xc[:])
                        # LN2 stats + center
                        st = st2p.tile([P, 4, 6], F32, tag="st2")
                        for j in range(4):
                            nc.vector.bn_stats(
                                st[:, j, :], x1t[:, j * 512 : (j + 1) * 512])
                        mv = st2p.tile([P, 2], F32, tag="mv2")
                        nc.vector.bn_aggr(mv[:], st[:])
                        rstd = st2p.tile([P, 1], F32, tag="rstd2")
                        nc.scalar.activation(
                            rstd[:], mv[:, 1:2], AF.Sqrt, bias=eps_sb[:])
                        nc.vector.reciprocal(rstd[:], rstd[:])
                        nmr = st2p.tile([P, 1], F32, tag="nmr2")
                        nc.vector.tensor_mul(nmr[:], mv[:, 0:1], rstd[:])
                        nc.vector.tensor_scalar_mul(nmr[:], nmr[:], -1.0)
                        q2t = qn2tp.tile([P, DIM], BF16, tag="qn2")
                        nc.scalar.activation(
                            q2t[:], x1t[:], AF.Identity,
                            bias=nmr[:], scale=rstd[:])
                        q2ts.append(q2t)
                        nc.sync.dma_start(
                            out_d[ts_ * P : (ts_ + 1) * P, :], x1t[:])
                    for c in range(DC):
                        pt = ps_tr.tile([P, 256], BF16, tag="tr")
                        for t2 in range(2):
                            nc.tensor.transpose(
                                pt[:, t2 * P : (t2 + 1) * P],
                                q2ts[t2][:, c * P : (c + 1) * P], identb[:])
                        dst = qn2T8[c // 2][:, c % 2,
                                           grp * 256 : (grp + 1) * 256]
                        with nc.allow_low_precision(reason="fp8 ffn inputs"):
                            if c % 2 == 0:
                                nc.scalar.activation(
                                    dst, pt[:],
                                    AF.Identity, bias=b2_sb[:, c : c + 1],
                                    scale=g2_sb[:, c : c + 1])
                            else:
                                nc.vector.tensor_scalar(
                                    dst, pt[:],
                                    scalar1=g2_sb[:, c : c + 1],
                                    scalar2=b2_sb[:, c : c + 1],
                                    op0=mybir.AluOpType.mult,
                                    op1=mybir.AluOpType.add)

        es_bw.close()
        es_qnT.close()
        es_qa.close()
        es_ao.close()
        es_wo.close()
        es_kv.close()

        # ---- Phase G: FFN1 (fp8 DoubleRow) -> h1b (fp8, packed) ---------
        # h1b[p, g8, r, t] = gelu(h1) at ffn-row g8*256 + r*128 + p
        es_h1 = ExitStack()
        h1p = es_h1.enter_context(tc.tile_pool(name="h1_pool", bufs=1))
        h1b = h1p.tile([P, FC // 2, 2, T], F8)
        with tc.tile_pool(name="w1_st", bufs=4) as w1st, \
             tc.tile_pool(name="ps_g", bufs=4, space="PSUM") as ps_g:
            for f in range(FC):
                w1t = w1st.tile([P, DC // 2, 2, P], F8, tag="w1")
                nc.sync.dma_start(
                    w1t[:],
                    w1_d[f * P : (f + 1) * P, :].rearrange(
                        "p (c8 r q) -> p c8 r q", r=2, q=P))
                for th in range(2):
                    pg = ps_g.tile([P, 512], F32, tag="g")
                    for c8 in range(DC // 2):
                        nc.tensor.matmul(
                            pg[:], w1t[:, c8, :, :],
                            qn2T8[c8][:, :, th * 512 : (th + 1) * 512],
                            start=(c8 == 0), stop=(c8 == DC // 2 - 1),
                            perf_mode=mybir.MatmulPerfMode.DoubleRow)
                    with nc.allow_low_precision(reason="fp8 ffn h1"):
                        nc.scalar.activation(
                            h1b[:, f // 2, f % 2,
                                th * 512 : (th + 1) * 512],
                            pg[:], AF.Gelu, scale=1.0 / W8SCALE)

        # ---- Phase H: FFN2 + gated residual accumulate ------------------
        with tc.tile_pool(name="w2_st", bufs=40) as w2st, \
             tc.tile_pool(name="outst", bufs=4) as outp, \
             tc.tile_pool(name="ps_f2", bufs=4, space="PSUM") as ps_f2:
            for dc4 in range(4):
                sl = slice(dc4 * 512, (dc4 + 1) * 512)
                # buffer all 32 W2 chunks for this output slice, then run
                # ts-outer so each token tile finishes (and stores) early
                w2ts = []
                for g8 in range(32):
                    w2t = w2st.tile([P, 2, 512], F8, tag="w2",
                                    name=f"w2t_{dc4}_{g8}")
                    nc.sync.dma_start(
                        w2t[:],
                        w2_d[(dc4 * 32 + g8) * P : (dc4 * 32 + g8 + 1) * P, :]
                        .rearrange("p (r d) -> p r d", d=512))
                    w2ts.append(w2t)
                for ts_ in range(TS):
                    pos = ps_f2.tile([P, 512], F32, tag="f2")
                    for g8 in range(32):
                        nc.tensor.matmul(
                            pos[:],
                            h1b[:, g8, :, ts_ * P : (ts_ + 1) * P],
                            w2ts[g8][:],
                            start=(g8 == 0), stop=(g8 == 31),
                            perf_mode=mybir.MatmulPerfMode.DoubleRow)
                    ot = outp.tile([P, 512], F32, tag="out")
                    if ts_ % 2 == 0:
                        nc.scalar.mul(ot[:], pos[:], tg2_sb[:])
                    else:
                        nc.vector.tensor_scalar(
                            ot[:], pos[:], scalar1=tg2_sb[:],
                            scalar2=None, op0=mybir.AluOpType.mult)
                    # accumulate the gated FFN output onto the residual
                    # already sitting in out_d (written during EF)
                    nc.gpsimd.dma_start(
                        out_d[ts_ * P : (ts_ + 1) * P, sl], ot[:],
                        accum_op=mybir.AluOpType.add)
        es_h1.close()
        es_qn2.close()

    nc.compile()
    return nc


_CACHED = None


def _get_program():
    global _CACHED
    if _CACHED is None:
        _CACHED = build_program()
    return _CACHED


def _prep_weights(inputs):
    wq = np.asarray(inputs["Wq"], dtype=np.float32)
    wkv = np.asarray(inputs["Wkv"], dtype=np.float32)
    wo = np.asarray(inputs["Wo"], dtype=np.float32)
    w1 = np.asarray(inputs["W1"], dtype=np.float32)
    w2 = np.asarray(inputs["W2"], dtype=np.float32)
    g1 = np.asarray(inputs["ln_q_g"], dtype=np.float32)
    b1 = np.asarray(inputs["ln_q_b"], dtype=np.float32)
    g2 = np.asarray(inputs["ln_ff_g"], dtype=np.float32)
    b2 = np.asarray(inputs["ln_ff_b"], dtype=np.float32)

    # wq8[c8*128+p, r*1024+q] = 32*Wq[c8*256+r*128+p, q]
    wq_bf = ((wq * W8SCALE).reshape(8, 2, P, INNER)
             .transpose(0, 2, 1, 3).astype(np_f8).reshape(8 * P, 2 * INNER))
    wkv3 = wkv.reshape(MC, P, 2 * INNER)
    wk_prep = (wkv3[:, :, :INNER].reshape(MC, P, IC, P)
               .transpose(2, 1, 0, 3).astype(np_bf16).reshape(IC * P, MC * P))
    wv_prep = (wkv3[:, :, INNER:].reshape(MC, P, 2, 512)
               .transpose(2, 1, 0, 3).astype(np_bf16).reshape(2 * P, MC * 512))
    # wo8[p, i8*2*DIM + r*DIM + d] = 32*Wo[i8*256 + r*128 + p, d]
    wo_prep = ((wo * W8SCALE).reshape(4, 2, P, DIM).transpose(2, 0, 1, 3)
               .astype(np_f8).reshape(P, 4 * 2 * DIM))
    # w1_prep[f*128+p, c8*256+r*128+q] = 32*W1[c8*256+r*128+p, f*128+q]
    w1_prep = ((w1 * W8SCALE).reshape(8, 2, P, FC, P)
               .transpose(3, 2, 0, 1, 4).astype(np_f8).reshape(FC * P, 2048))
    # w2_prep[(dc4*32+g8)*128+p, r*512+d] = 32*W2[g8*256+r*128+p, dc4*512+d]
    w2_prep = ((w2 * W8SCALE).reshape(32, 2, P, 4, 512)
               .transpose(3, 0, 2, 1, 4).astype(np_f8).reshape(128 * P, 1024))
    g1s = np.ascontiguousarray(g1.reshape(DC, P).T)
    b1s = np.ascontiguousarray(b1.reshape(DC, P).T)
    g2p = np.ascontiguousarray(g2.reshape(DC, P).T)
    b2p = np.ascontiguousarray(b2.reshape(DC, P).T)
    tg1 = (np.tanh(np.asarray(inputs["attn_gate"], dtype=np.float32))
           / W8SCALE).reshape(1, 1)
    tg2 = (np.tanh(np.asarray(inputs["ff_gate"], dtype=np.float32))
           / W8SCALE).reshape(1, 1)
    sumsel_np = np.zeros((P, 2), dtype=np_bf16)
    sumsel_np[:LAT, 0] = 1.0
    sumsel_np[LAT:, 1] = 1.0
    onehot_np = np.ascontiguousarray(sumsel_np.T)
    return {
        "Wq": wq_bf, "Wk": wk_prep, "Wv": wv_prep, "Wo": wo_prep,
        "W1": w1_prep, "W2": w2_prep,
        "g1s": g1s, "b1s": b1s, "g2": g2p, "b2": b2p,
        "tg1": tg1, "tg2": tg2,
        "sumsel": sumsel_np, "onehot": onehot_np,
    }


def kernel(**inputs):
    x = np.asarray(inputs["x"], dtype=np.float32)
    media = np.asarray(inputs["media"], dtype=np.float32)
    mask = np.asarray(inputs["media_mask"])
    shared = _prep_weights(inputs)

    nc = _get_program()
    in_maps = []
    for core in range(NCORES):
        b = core // 2
        half = core % 2
        masklog = np.where(mask[b], 0.0, -50.0).astype(np.float32).reshape(LAT, 1)
        in_maps.append({
            "x": np.ascontiguousarray(
                x[b, half * T : (half + 1) * T, :]).astype(np_bf16),
            "mediaT": np.ascontiguousarray(
                media[b].reshape(LAT, MC, P).transpose(2, 1, 0)
            ).astype(np_bf16).reshape(P, MC * LAT),
            "masklog": masklog,
            **shared,
        })
    res = run_bass_kernel_spmd(nc, in_maps, core_ids=list(range(NCORES)))
    out = np.empty((B, NTOK, DIM), dtype=np.float32)
    for core in range(NCORES):
        b = core // 2
        half = core % 2
        out[b, half * T : (half + 1) * T, :] = res.results[core]["out"]
    return out
